# revision 1
# baseline (speedup 1.0000x reference)
"""Trainium2 Bass kernel for a 12-head causal attention block (GPT-2 style).

Problem: x:[4,2048,768] -> qkv = x@W_attn+b_attn, causal softmax attention
(12 heads, d=64), out @ W_proj + b_proj.

Sharding over 8 NeuronCores: core c handles batch b=c//2 (data parallel) and
head-group hg=c%2 (6 heads, tensor parallel on the qkv columns / proj rows).
Each core returns a partial projection output; the host sums the two
head-group partials per batch and adds b_proj.

Per-core dataflow (all matmuls in float32r: full speed, ~1e-3 rel err):
  - x [2048,768] is PE-transposed to xT (emb on partitions).
  - qkvT = W-tiles.T @ xT  -> qT,kT per head-pair [128,2048]; v is computed
    non-transposed (v = xT-tiles.T @ Wv) since P@V needs V with seq on
    partitions.  b_attn folded in (per-partition add for q/k, rank-1 matmul
    for v).
  - scores S^T[k,q] per 128k x 512q block: lhsT=kT[d=64 rows], rhs=qT.  The
    two heads of a pair run row-packed (tile_position (0,0)/(64,0)) writing
    adjacent PSUM banks, so one ACT exp call covers both heads.
  - causal: upper-triangle blocks are skipped entirely; the diagonal-crossing
    128x128 triangle is zeroed post-exp with gpsimd affine_select.  Softmax
    needs no max-subtraction here (|scores/8| < ~4, exp is safe in fp32).
  - P@V and the softmax denominators accumulate in PSUM over k-tiles:
    AV col-packed per head pair; the denominator matmul uses an all-ones
    [128,64] lhsT so the sums land already broadcast across 64 partitions;
    DVE reciprocal + multiply then writes normalized attn-out^T to SBUF.
  - proj: y[128q,768] accumulated over the 3 head-pair k-tiles, DMA'd out
    straight from PSUM.
"""

import os
import ml_dtypes
import numpy as np

N_HEAD = 12
N_EMBD = 768
HEAD_DIM = 64
B, S = 4, 2048
N_CORES = 8
HG_HEADS = 6            # heads per core (3 pairs)
HG_DIM = HG_HEADS * HEAD_DIM   # 384
QKV_W = 3 * HG_DIM      # 1152 qkv columns per core
N_PAIRS = 3
ST = S // 128           # 16 seq tiles of 128
NG = S // 512           # 4 seq groups of 512

# last run's BassKernelResults (test.py reads this for HW timing / traces)
LAST_RESULTS = None
_PROGRAM = None


def _build_program(loop_n=None, skip=()):
    """Build (once) the SPMD Bass program run identically on all 8 cores.

    skip: benchmark-only ablation flags ({"xT","qkv","attn","proj","act",
    "tri","norm","dma_in"}) — disable pieces to attribute time; output is
    garbage when used.
    loop_n: benchmark mode — inputs become internal DRAM tensors (no host
    transfer) and the whole kernel body repeats loop_n times in a hardware
    loop, so per-iteration time can be measured as a slope between two
    loop counts (the axon tunnel's dispatch/transfer jitter cancels).
    """
    import concourse.bacc as bacc
    import concourse.tile as tile
    from concourse import mybir, masks

    F32R = mybir.dt.float32r
    F32 = mybir.dt.float32
    BF16 = mybir.dt.bfloat16
    AF = mybir.ActivationFunctionType

    nc = bacc.Bacc(None, target_bir_lowering=False)
    if loop_n is not None:
        dummy_d = nc.declare_dram_parameter("bench_in", [1, 128], F32, isOutput=False)
        tout_d = nc.declare_dram_parameter("bench_out", [1, 128], F32, isOutput=True)
        x_d = nc.dram_tensor("x", [S, N_EMBD], F32)
        wqkv_d = nc.dram_tensor("w_qkv", [N_EMBD, QKV_W], F32R)
        bqk_d = nc.dram_tensor("b_qk", [768], F32)
        bv_d = nc.dram_tensor("b_v", [HG_DIM], F32R)
        wproj_d = nc.dram_tensor("w_proj", [HG_DIM, N_EMBD], F32R)
        ones_d = nc.dram_tensor("ones", [1, 128], F32R)
        y_d = nc.dram_tensor("y", [S, N_EMBD], F32)
    else:
        x_d = nc.declare_dram_parameter("x", [S, N_EMBD], F32, isOutput=False)
        wqkv_d = nc.declare_dram_parameter("w_qkv", [N_EMBD, QKV_W], F32R, isOutput=False)
        bqk_d = nc.declare_dram_parameter("b_qk", [768], F32, isOutput=False)
        bv_d = nc.declare_dram_parameter("b_v", [HG_DIM], F32R, isOutput=False)
        wproj_d = nc.declare_dram_parameter("w_proj", [HG_DIM, N_EMBD], F32R, isOutput=False)
        ones_d = nc.declare_dram_parameter("ones", [1, 128], F32R, isOutput=False)
        y_d = nc.declare_dram_parameter("y", [S, N_EMBD], F32, isOutput=True)

    with tile.TileContext(nc) as tc:
        from contextlib import ExitStack

        with ExitStack() as outer:
            if loop_n is not None:
                outer.enter_context(tc.For_i(0, loop_n, 1))
            consts = outer.enter_context(tc.tile_pool(name="consts", bufs=1))
            ident = consts.tile([128, 128], F32)
            masks.make_identity(nc, ident[:])
            ones_row = consts.tile([1, 128], F32R)    # v-bias rank-1 lhsT
            nc.sync.dma_start(out=ones_row[:], in_=ones_d[:])
            bias_qk = consts.tile([128, 6], F32)      # col m: b_qk[128m:128m+128]
            nc.sync.dma_start(
                out=bias_qk[:], in_=bqk_d[0:768].rearrange("(m p) -> p m", p=128)
            )
            bias_v = consts.tile([1, HG_DIM], F32R)
            nc.sync.dma_start(
                out=bias_v[:], in_=bv_d[0:HG_DIM].rearrange("(o v) -> o v", o=1)
            )

            # ---- persistent activations/weights in SBUF ----
            big = outer.enter_context(tc.tile_pool(name="big", bufs=1))
            xT = big.tile([128, 6 * S], F32R)      # [emb-part, k-tile*2048+seq]
            qkT = big.tile([128, 6 * S], BF16)     # m=0..2 qT pairs, m=3..5 kT pairs
            q_odd = big.tile([64, N_PAIRS * S], BF16)  # odd heads shifted to base 0
            k_odd = big.tile([64, N_PAIRS * S], BF16)
            # per k-tile: 6 heads x (64 v-cols + a ones col for the softmax
            # denominator) -> P@V and row-sums come from one M=65 matmul
            v_all = big.tile([128, ST * 390], BF16)  # [seq, t*390 + 65h + d]
            nc.gpsimd.memset(v_all[:], 1.0)
            attnT = big.tile([128, N_PAIRS * S], F32R)  # [pair d, pair*2048+seq]
            w_proj = big.tile([128, N_PAIRS * N_EMBD], F32R)
            for p in range(N_PAIRS):
                nc.sync.dma_start(
                    out=w_proj[:, p * N_EMBD:(p + 1) * N_EMBD],
                    in_=wproj_d[p * 128:(p + 1) * 128, :],
                )

            if "qkv" in skip and "attn" not in skip:
                # seed reads of otherwise-unwritten tensors (bench ablation)
                nc.sync.dma_start(out=qkT[0:1, 0:128],
                                  in_=ones_d[:].bitcast(BF16)[:, 0:128])
                nc.sync.dma_start(out=v_all[0:1, 0:128],
                                  in_=ones_d[:].bitcast(BF16)[:, 0:128])

            # ---- phase A: load x tiles + PE-transpose into xT ----
            with tc.tile_pool(name="xload", bufs=3) as xload, \
                 tc.tile_pool(name="tps", bufs=2, space="PSUM") as tps:
                xT_v = xT[:].rearrange("p (k s) -> p k s", k=6)
                for t in range(ST if "xT" not in skip else 0):
                    xs = xload.tile([128, N_EMBD], F32)
                    if "dma_in" not in skip:
                        nc.sync.dma_start(out=xs[:], in_=x_d[t * 128:(t + 1) * 128, :])
                    tp = tps.tile([128, N_EMBD], F32)
                    for k in range(6):
                        nc.tensor.transpose(
                            tp[:, k * 128:(k + 1) * 128],
                            xs[:, k * 128:(k + 1) * 128],
                            ident[:],
                        )
                    nc.vector.tensor_copy(
                        xT_v[:, :, t * 128:(t + 1) * 128],
                        tp[:].rearrange("p (k s) -> p k s", k=6),
                    )

            # ---- phase B: qkv projections ----
            with tc.tile_pool(name="wqkv", bufs=1) as wq_pool, \
                 tc.tile_pool(name="qkps", bufs=4, space="PSUM") as qkps, \
                 tc.tile_pool(name="vps", bufs=2, space="PSUM") as vps:
                w_all = wq_pool.tile([128, 6 * QKV_W], F32R)
                for k in range(6 if "dma_in" not in skip else 0):
                    nc.sync.dma_start(
                        out=w_all[:, k * QKV_W:(k + 1) * QKV_W],
                        in_=wqkv_d[k * 128:(k + 1) * 128, :],
                    )
                # q/k: transposed layout -> qkT
                for m in range(6 if "qkv" not in skip else 0):
                    for g in range(NG):
                        ps = qkps.tile([128, 512], F32)
                        for k in range(6):
                            nc.tensor.matmul(
                                ps[:],
                                w_all[:, k * QKV_W + m * 128:k * QKV_W + (m + 1) * 128],
                                xT[:, k * S + g * 512:k * S + g * 512 + 512],
                                start=(k == 0), stop=(k == 5),
                            )
                        nc.vector.tensor_scalar_add(
                            qkT[:, m * S + g * 512:m * S + g * 512 + 512],
                            ps[:], bias_qk[:, m:m + 1],
                        )
                # v: natural [seq, d] layout, interleaved with ones columns
                v_v = v_all[:].rearrange("p (t c) -> p t c", t=ST)
                for t in range(ST if "qkv" not in skip else 0):
                    ps = vps.tile([128, HG_DIM], F32)
                    for k in range(6):
                        nc.tensor.matmul(
                            ps[:],
                            xT[:, k * S + t * 128:k * S + (t + 1) * 128],
                            w_all[:, k * QKV_W + 768:k * QKV_W + QKV_W],
                            start=(k == 0), stop=False,
                        )
                    nc.tensor.matmul(   # += ones^T[1,128].T @ bias_v[1,384]
                        ps[:], ones_row[:], bias_v[:], start=False, stop=True,
                    )
                    nc.vector.tensor_copy(
                        v_v[:, t, :].rearrange("p (h c) -> p h c", h=6)[:, :, 0:64],
                        ps[:].rearrange("p (h d) -> p h d", h=6),
                    )

            # ---- phase C: causal attention, one head-pair at a time ----
            # odd heads' qT/kT shifted to partition base 0 (SBUF->SBUF DMA);
            # a matmul lhsT/rhs at base partition 64 crashes at runtime.
            for pair in range(N_PAIRS if "attn" not in skip else 0):
                nc.sync.dma_start(
                    out=q_odd[:, pair * S:(pair + 1) * S],
                    in_=qkT[64:128, pair * S:(pair + 1) * S])
                nc.sync.dma_start(
                    out=k_odd[:, pair * S:(pair + 1) * S],
                    in_=qkT[64:128, (3 + pair) * S:(4 + pair) * S])
            with tc.tile_pool(name="stps", bufs=2, space="PSUM") as stps, \
                 tc.tile_pool(name="avps", bufs=3, space="PSUM") as avps, \
                 tc.tile_pool(name="bcps", bufs=1, space="PSUM") as bcps, \
                 tc.tile_pool(name="ptp", bufs=3) as ptp, \
                 tc.tile_pool(name="rcp", bufs=2) as rcp, \
                 tc.tile_pool(name="bcsb", bufs=2) as bcsb, \
                 tc.tile_pool(name="shtmp", bufs=2) as shtmp:
                for pair in range(N_PAIRS if "attn" not in skip else 0):
                    q0 = pair * S          # qT pair tile offset in qkT
                    k0 = (3 + pair) * S    # kT pair tile offset
                    for g in range(NG):
                        av0 = avps.tile([65, 512], F32, tag="av")
                        av1 = avps.tile([65, 512], F32, tag="av")
                        avs = (av0, av1)
                        njt = 4 * g + 4
                        for j in range(njt):
                            diag_r = j - 4 * g   # >=0 on diagonal-crossing tiles
                            c0 = 128 * diag_r if diag_r >= 0 else 0
                            st = stps.tile([128, 1024], F32)   # h1 | h2
                            pt = ptp.tile([128, 1024], BF16)
                            if "scores" in skip:
                                continue
                            nc.tensor.matmul(
                                st[:, c0:512],
                                qkT[0:64, k0 + j * 128:k0 + (j + 1) * 128],
                                qkT[0:64, q0 + g * 512 + c0:q0 + (g + 1) * 512],
                                start=True, stop=True,
                            )
                            nc.tensor.matmul(
                                st[:, 512 + c0:1024],
                                k_odd[:, q0 + j * 128:q0 + (j + 1) * 128],
                                q_odd[:, q0 + g * 512 + c0:q0 + (g + 1) * 512],
                                start=True, stop=True,
                            )
                            # exp(S/8) over both heads' valid columns
                            if "act_small" in skip:
                                nc.scalar.activation(
                                    pt[:, c0:c0 + 64], st[:, c0:c0 + 64],
                                    AF.Exp, bias=0.0, scale=0.125,
                                )
                            else:
                                nc.scalar.activation(
                                    pt[:, c0:1024], st[:, c0:1024], AF.Exp,
                                    bias=0.0, scale=0.125,
                                )
                            if diag_r >= 0 and "tri" not in skip:
                                # zero the strictly-lower (k>q) triangle
                                for h in range(2):
                                    nc.gpsimd.affine_select(
                                        out=pt[:, h * 512 + c0:h * 512 + c0 + 128],
                                        in_=pt[:, h * 512 + c0:h * 512 + c0 + 128],
                                        compare_op=mybir.AluOpType.is_ge,
                                        fill=0.0, base=0,
                                        pattern=[[1, 128]], channel_multiplier=-1,
                                    )
                            first, last = (j == 0), (j == njt - 1)
                            for h in range(2):
                                hl = 2 * pair + h
                                nc.tensor.matmul(   # [attn-out^T ; denominators]
                                    avs[h][0:65, c0:512],
                                    v_all[:, j * 390 + hl * 65:j * 390 + hl * 65 + 65],
                                    pt[:, h * 512 + c0:(h + 1) * 512],
                                    start=first, stop=last,
                                )
                        nw = 64 if "norm_small" in skip else 512
                        cols = slice(pair * S + g * 512, pair * S + g * 512 + nw)
                        for h in range(2):
                            rc_row = rcp.tile([1, 512], F32R)
                            with nc.allow_low_precision(reason="f32r recip feeds matmul"):
                                nc.vector.reciprocal(rc_row[:, :nw],
                                                     avs[h][64:65, :nw])
                            bc = bcps.tile([64, 512], F32)
                            nc.tensor.matmul(bc[:, :nw], ones_row[:, 0:64],
                                             rc_row[:, :nw], start=True, stop=True)
                            bc_sb = bcsb.tile([64, 512], F32)
                            nc.vector.tensor_copy(bc_sb[:, :nw], bc[:, :nw])
                            if h == 0:
                                nc.vector.tensor_mul(
                                    attnT[0:64, cols], avs[h][0:64, :nw],
                                    bc_sb[:, :nw])
                            else:
                                # DVE lanes are partition-locked: odd head's
                                # rows 64-127 go via an SBUF bounce + DMA shift
                                tmp = shtmp.tile([64, 512], F32R)
                                nc.vector.tensor_mul(
                                    tmp[:, :nw], avs[h][0:64, :nw], bc_sb[:, :nw])
                                nc.sync.dma_start(out=attnT[64:128, cols],
                                                  in_=tmp[:, :nw])

            # ---- phase D: output projection (partial; host adds b_proj) ----
            with tc.tile_pool(name="yps", bufs=3, space="PSUM") as yps, \
                 tc.tile_pool(name="ystage", bufs=3) as ystage:
                for t in range(ST if "proj" not in skip else 0):
                    ps = yps.tile([128, N_EMBD], F32)
                    for p in range(N_PAIRS):
                        for h0, hw in ((0, 512), (512, 256)):
                            nc.tensor.matmul(
                                ps[:, h0:h0 + hw],
                                attnT[:, p * S + t * 128:p * S + (t + 1) * 128],
                                w_proj[:, p * N_EMBD + h0:p * N_EMBD + h0 + hw],
                                start=(p == 0), stop=(p == N_PAIRS - 1),
                            )
                    ys = ystage.tile([128, N_EMBD], F32)
                    nc.vector.tensor_copy(ys[:], ps[:])
                    nc.sync.dma_start(out=y_d[t * 128:(t + 1) * 128, :], in_=ys[:])

        if loop_n is not None:
            nc.sync.dma_start(out=tout_d[:], in_=dummy_d[:])

    nc.compile()
    return nc


def _numpy_fallback(x, mask, W_attn, b_attn, W_proj, b_proj):
    qkv = x @ W_attn + b_attn
    q, k, v = np.split(qkv, 3, axis=-1)

    def heads(t):
        return t.reshape(B, S, N_HEAD, HEAD_DIM).transpose(0, 2, 1, 3)

    q, k, v = heads(q), heads(k), heads(v)
    attn = np.einsum("bhqd,bhkd->bhqk", q, k) / np.sqrt(np.float32(HEAD_DIM))
    attn = attn + mask * (-1e9)
    attn = attn - attn.max(axis=-1, keepdims=True)
    attn = np.exp(attn)
    attn = attn / attn.sum(axis=-1, keepdims=True)
    out = np.einsum("bhqk,bhkd->bhqd", attn, v)
    out = out.transpose(0, 2, 1, 3).reshape(B, S, N_EMBD)
    return (out @ W_proj + b_proj).astype(np.float32)


def kernel(x, mask, W_attn, b_attn, W_proj, b_proj):
    global LAST_RESULTS, _PROGRAM
    x = np.asarray(x, dtype=np.float32)
    mask = np.asarray(mask, dtype=np.float32)
    W_attn = np.asarray(W_attn, dtype=np.float32)
    b_attn = np.asarray(b_attn, dtype=np.float32)
    W_proj = np.asarray(W_proj, dtype=np.float32)
    b_proj = np.asarray(b_proj, dtype=np.float32)

    # the kernel exploits causal structure; verify the mask actually is causal
    causal = 1.0 - np.tril(np.ones((S, S), dtype=np.float32))
    if mask.shape != (1, 1, S, S) or not np.array_equal(mask[0, 0], causal):
        return _numpy_fallback(x, mask, W_attn, b_attn, W_proj, b_proj)

    from concourse.bass_utils import run_bass_kernel_spmd

    if _PROGRAM is None:
        _PROGRAM = _build_program()

    in_maps = make_in_maps(x, W_attn, b_attn, W_proj)

    trace = bool(int(os.environ.get("ATTN_KERNEL_TRACE", "0")))
    res = run_bass_kernel_spmd(_PROGRAM, in_maps, list(range(N_CORES)), trace=trace)
    LAST_RESULTS = res

    y = np.zeros((B, S, N_EMBD), dtype=np.float32)
    for c in range(N_CORES):
        y[c // 2] += res.results[c]["y"]
    y += b_proj
    return y


def make_in_maps(x, W_attn, b_attn, W_proj):
    in_maps = []
    for c in range(N_CORES):
        b, hg = divmod(c, 2)
        o = HG_DIM * hg
        in_maps.append({
            "x": np.ascontiguousarray(x[b]),
            "w_qkv": np.ascontiguousarray(np.concatenate(
                [W_attn[:, o:o + HG_DIM],
                 W_attn[:, 768 + o:768 + o + HG_DIM],
                 W_attn[:, 1536 + o:1536 + o + HG_DIM]], axis=1)),
            "b_qk": np.ascontiguousarray(np.concatenate(
                [b_attn[o:o + HG_DIM], b_attn[768 + o:768 + o + HG_DIM]])),
            "b_v": np.ascontiguousarray(b_attn[1536 + o:1536 + o + HG_DIM]),
            "w_proj": np.ascontiguousarray(W_proj[o:o + HG_DIM, :]),
            "ones": np.ones((1, 128), dtype=np.float32),
        })
    return in_maps



# revision 2
# speedup vs baseline: 1.3216x; 1.3216x over previous
"""Trainium2 Bass kernel for a 12-head causal attention block (GPT-2 style).

Problem: x:[4,2048,768] -> qkv = x@W_attn+b_attn, causal softmax attention
(12 heads, d=64), out @ W_proj + b_proj.

Sharding over 8 NeuronCores: core c handles batch b=c//2 (data parallel) and
head-group hg=c%2 (6 heads = 3 head-pairs, tensor parallel on the qkv
columns / proj rows).  Each core returns a partial projection output; the
host sums the two head-group partials per batch and adds b_proj.

v2 design (vs the 410us baseline):
  - x is transposed and bf16-cast on the HOST: no PE-transpose phase, half
    the input DMA bytes, and every matmul operand is bf16 (fast weight
    load applies; PSUM accumulation stays fp32).
  - scores: the two heads of a pair run CONCURRENTLY in the PE array via
    row tiling (tile_position (0,0)/(64,0), K=64 each) - halves score time.
  - per-(pair,g) attention group: j-loop over k-tiles software-pipelined
    one stage deep (scores j+1 emitted before AV j) so the ACT exp of tile
    j overlaps the scores matmul of j+1.
  - AV uses the M=65 ones-column trick: attention output AND softmax
    denominators from one accumulating matmul per head.
  - normalization: DVE reciprocal -> GPSIMD partition_broadcast -> DVE
    multiply (no PE broadcast matmul, no PSUM->SBUF bounce of it).
  - qkv / proj matmuls are emitted as small work units INTERLEAVED into the
    attention j-loops: the PE executes them while ACT (the per-group
    bottleneck at ~1 elem/cycle/lane) chews on exp, keeping the PE dense so
    the HAM clock stays at 2.4 GHz.
  - PSUM budget: scores 2x[128,1024] (4 banks) + AV 2x[65,512] (2 banks) +
    shared aux pool 2x[128,512] (2 banks) = 8 banks exactly.
"""

import os
import ml_dtypes
import numpy as np

N_HEAD = 12
N_EMBD = 768
HEAD_DIM = 64
B, S = 4, 2048
N_CORES = 8
HG_HEADS = 6            # heads per core (3 pairs)
HG_DIM = HG_HEADS * HEAD_DIM   # 384
QKV_W = 3 * HG_DIM      # 1152 qkv columns per core
N_PAIRS = 3
ST = S // 128           # 16 seq tiles of 128
NG = S // 512           # 4 seq groups of 512

LAST_RESULTS = None
_PROGRAM = None


def _build_program():
    import concourse.bacc as bacc
    import concourse.tile as tile
    from concourse import mybir

    F32 = mybir.dt.float32
    BF16 = mybir.dt.bfloat16
    AF = mybir.ActivationFunctionType

    nc = bacc.Bacc(None, target_bir_lowering=False)
    xT_d = nc.declare_dram_parameter("xT", [N_EMBD, S], BF16, isOutput=False)
    wqkv_d = nc.declare_dram_parameter("w_qkv", [N_EMBD, QKV_W], BF16, isOutput=False)
    bqk_d = nc.declare_dram_parameter("b_qk", [768], F32, isOutput=False)
    bv_d = nc.declare_dram_parameter("b_v", [HG_DIM], BF16, isOutput=False)
    wproj_d = nc.declare_dram_parameter("w_proj", [HG_DIM, N_EMBD], BF16, isOutput=False)
    ones_d = nc.declare_dram_parameter("ones", [1, 128], BF16, isOutput=False)
    y_d = nc.declare_dram_parameter("y", [S, N_EMBD], F32, isOutput=True)

    with tile.TileContext(nc) as tc:
        from contextlib import ExitStack

        with ExitStack() as outer:
            consts = outer.enter_context(tc.tile_pool(name="consts", bufs=1))
            ones_row = consts.tile([1, 128], BF16)
            nc.sync.dma_start(out=ones_row[:], in_=ones_d[:])
            bias_qk = consts.tile([128, 6], F32)      # col m: b_qk[128m:128m+128]
            nc.sync.dma_start(
                out=bias_qk[:], in_=bqk_d[0:768].rearrange("(m p) -> p m", p=128)
            )
            bias_v = consts.tile([1, HG_DIM], BF16)
            nc.sync.dma_start(
                out=bias_v[:], in_=bv_d[0:HG_DIM].rearrange("(o v) -> o v", o=1)
            )

            # ---- persistent activations/weights in SBUF (all bf16) ----
            big = outer.enter_context(tc.tile_pool(name="big", bufs=1))
            xT = big.tile([128, 6 * S], BF16)       # [emb-part, k-chunk*2048+seq]
            w_all = big.tile([128, 6 * QKV_W], BF16)
            w_proj = big.tile([128, N_PAIRS * N_EMBD], BF16)
            qkT = big.tile([128, 6 * S], BF16)      # m=0..2 qT pairs, m=3..5 kT pairs
            # per k-tile: 6 heads x (64 v-cols + a ones col for the softmax
            # denominator) -> P@V and row-sums come from one M=65 matmul
            v_all = big.tile([128, ST * 390], BF16)  # [seq, t*390 + 65h + d]
            attnT = big.tile([128, N_PAIRS * S], BF16)

            nc.gpsimd.memset(v_all[:], 1.0)
            for k in range(6):
                nc.sync.dma_start(out=xT[:, k * S:(k + 1) * S],
                                  in_=xT_d[k * 128:(k + 1) * 128, :])
                nc.sync.dma_start(out=w_all[:, k * QKV_W:(k + 1) * QKV_W],
                                  in_=wqkv_d[k * 128:(k + 1) * 128, :])
            for p in range(N_PAIRS):
                nc.sync.dma_start(out=w_proj[:, p * N_EMBD:(p + 1) * N_EMBD],
                                  in_=wproj_d[p * 128:(p + 1) * 128, :])

            # ---- pools ----
            stps = outer.enter_context(tc.tile_pool(name="stps", bufs=2, space="PSUM"))
            avps = outer.enter_context(tc.tile_pool(name="avps", bufs=2, space="PSUM"))
            auxps = outer.enter_context(tc.tile_pool(name="auxps", bufs=2, space="PSUM"))
            ptp = outer.enter_context(tc.tile_pool(name="ptp", bufs=3))
            rcp = outer.enter_context(tc.tile_pool(name="rcp", bufs=4))
            bcp = outer.enter_context(tc.tile_pool(name="bcp", bufs=4))
            shtmp = outer.enter_context(tc.tile_pool(name="shtmp", bufs=2))
            ystage = outer.enter_context(tc.tile_pool(name="ystage", bufs=3))

            v_view = v_all[:].rearrange("p (t h c) -> p t h c", t=ST, h=HG_HEADS)

            # ---- work-unit emitters (each emits a small PE-dense chunk) ----
            def emit_qk_group(m, g):
                # qkT[:, m*S + g*512 : +512] = (W[:, m-block].T @ xT)[:, g-block] + bias
                ps = auxps.tile([128, 512], F32, tag="aux")
                for k in range(6):
                    nc.tensor.matmul(
                        ps[:],
                        w_all[:, k * QKV_W + m * 128:k * QKV_W + (m + 1) * 128],
                        xT[:, k * S + g * 512:k * S + (g + 1) * 512],
                        start=(k == 0), stop=(k == 5),
                    )
                nc.vector.tensor_scalar_add(
                    qkT[:, m * S + g * 512:m * S + (g + 1) * 512],
                    ps[:], bias_qk[:, m:m + 1],
                )

            def emit_v_tile(pair, t):
                # v rows t*128.. for this pair's two heads (N=128)
                ps = auxps.tile([128, 128], F32, tag="aux")
                wc0 = 768 + pair * 128
                for k in range(6):
                    nc.tensor.matmul(
                        ps[:],
                        xT[:, k * S + t * 128:k * S + (t + 1) * 128],
                        w_all[:, k * QKV_W + wc0:k * QKV_W + wc0 + 128],
                        start=(k == 0), stop=False,
                    )
                nc.tensor.matmul(   # += ones^T[1,128].T @ bias_v[1,128]
                    ps[:], ones_row[:], bias_v[:, pair * 128:(pair + 1) * 128],
                    start=False, stop=True,
                )
                nc.vector.tensor_copy(
                    v_view[:, t, 2 * pair:2 * pair + 2, 0:64],
                    ps[:].rearrange("p (h d) -> p h d", h=2),
                )

            def emit_proj_tile(t):
                psA = auxps.tile([128, 512], F32, tag="aux")
                psB = auxps.tile([128, 256], F32, tag="aux")
                for p in range(N_PAIRS):
                    lhsT = attnT[:, p * S + t * 128:p * S + (t + 1) * 128]
                    nc.tensor.matmul(psA[:], lhsT, w_proj[:, p * N_EMBD:p * N_EMBD + 512],
                                     start=(p == 0), stop=(p == N_PAIRS - 1))
                    nc.tensor.matmul(psB[:], lhsT,
                                     w_proj[:, p * N_EMBD + 512:(p + 1) * N_EMBD],
                                     start=(p == 0), stop=(p == N_PAIRS - 1))
                ys = ystage.tile([128, N_EMBD], F32)
                nc.vector.tensor_copy(ys[:, 0:512], psA[:])
                nc.vector.tensor_copy(ys[:, 512:768], psB[:])
                nc.sync.dma_start(out=y_d[t * 128:(t + 1) * 128, :], in_=ys[:])

            # ---- attention group with interleaved background units ----
            def emit_attn_group(pair, g, bg_units):
                """bg_units: list of 0-arg emitters pulled into PE slack."""
                q0 = pair * S
                k0 = (3 + pair) * S
                njt = 4 * g + 4
                av0 = avps.tile([65, 512], F32, tag="av")
                av1 = avps.tile([65, 512], F32, tag="av")
                sts = {}
                pts = {}

                def scores(j):
                    diag_r = j - 4 * g
                    c0 = 128 * diag_r if diag_r >= 0 else 0
                    st = stps.tile([128, 1024], F32, tag="st")
                    nc.tensor.matmul(
                        st[:, c0:512],
                        qkT[0:64, k0 + j * 128:k0 + (j + 1) * 128],
                        qkT[0:64, q0 + g * 512 + c0:q0 + (g + 1) * 512],
                        start=True, stop=True, tile_position=(0, 0),
                    )
                    nc.tensor.matmul(
                        st[:, 512 + c0:1024],
                        qkT[64:128, k0 + j * 128:k0 + (j + 1) * 128],
                        qkT[64:128, q0 + g * 512 + c0:q0 + (g + 1) * 512],
                        start=True, stop=True, tile_position=(64, 0),
                    )
                    sts[j] = (st, c0)

                def expmask(j):
                    st, c0 = sts.pop(j)
                    pt = ptp.tile([128, 1024], BF16, tag="pt")
                    nc.scalar.activation(pt[:, c0:1024], st[:, c0:1024],
                                         AF.Exp, bias=0.0, scale=0.125)
                    diag_r = j - 4 * g
                    if diag_r >= 0:
                        for h in range(2):
                            nc.gpsimd.affine_select(
                                out=pt[:, h * 512 + c0:h * 512 + c0 + 128],
                                in_=pt[:, h * 512 + c0:h * 512 + c0 + 128],
                                compare_op=mybir.AluOpType.is_ge,
                                fill=0.0, base=0,
                                pattern=[[1, 128]], channel_multiplier=-1,
                            )
                    pts[j] = (pt, c0)

                def av(j):
                    pt, c0 = pts.pop(j)
                    first, last = (j == 0), (j == njt - 1)
                    for h, avt in ((0, av0), (1, av1)):
                        nc.tensor.matmul(
                            avt[0:65, c0:512],
                            v_all[:, j * 390 + (2 * pair + h) * 65:
                                  j * 390 + (2 * pair + h) * 65 + 65],
                            pt[:, h * 512 + c0:(h + 1) * 512],
                            start=first, stop=last,
                        )

                # schedule bg unit pulls evenly across the j loop
                pulls = [0] * njt
                for i in range(len(bg_units)):
                    pulls[(i * njt) // len(bg_units)] += 1

                scores(0)
                expmask(0)
                for j in range(njt):
                    if j + 1 < njt:
                        scores(j + 1)
                        expmask(j + 1)
                    for _ in range(pulls[j]):
                        bg_units.pop(0)()
                    av(j)

                # normalize: recip -> gpsimd broadcast -> multiply
                cols = slice(pair * S + g * 512, pair * S + (g + 1) * 512)
                with nc.allow_low_precision(reason="softmax normalize in bf16"):
                    for h, avt in ((0, av0), (1, av1)):
                        rc = rcp.tile([1, 512], F32)
                        nc.vector.reciprocal(rc[:], avt[64:65, :])
                        bc = bcp.tile([64, 512], F32)
                        nc.gpsimd.partition_broadcast(bc[:], rc[:], channels=64)
                        if h == 0:
                            nc.vector.tensor_mul(attnT[0:64, cols], avt[0:64, :], bc[:])
                        else:
                            # DVE lanes are partition-locked: odd head's rows
                            # 64-127 go via an SBUF bounce + DMA shift
                            tmp = shtmp.tile([64, 512], BF16)
                            nc.vector.tensor_mul(tmp[:], avt[0:64, :], bc[:])
                            nc.sync.dma_start(out=attnT[64:128, cols], in_=tmp[:])

            # ================= schedule =================
            # upfront: just enough qkv for attn(0, g0)
            emit_qk_group(3, 0)          # kT pair 0, seq 0-511
            emit_qk_group(0, 0)          # qT pair 0, seq 0-511
            for t in range(4):
                emit_v_tile(0, t)

            def pair_bg(p):
                """qkv units for pair p beyond (g0 kT/qT, v t0-3), in need order."""
                units = []
                for g in range(1, NG):
                    units.append(lambda m=3 + p, g=g: emit_qk_group(m, g))
                    units.append(lambda m=p, g=g: emit_qk_group(m, g))
                    units.extend(lambda pair=p, t=t: emit_v_tile(pair, t)
                                 for t in range(4 * g, 4 * g + 4))
                return units

            def pair_head(p):
                """the upfront units of pair p (needed before attn(p, g0))."""
                units = [lambda m=3 + p: emit_qk_group(m, 0),
                         lambda m=p: emit_qk_group(m, 0)]
                units.extend(lambda pair=p, t=t: emit_v_tile(pair, t)
                             for t in range(4))
                return units

            # background queues per attention slot
            slot_bg = {
                0: pair_bg(0) + pair_head(1) + pair_bg(1),
                1: pair_head(2) + pair_bg(2),
                2: [],   # proj units are appended per-group below
            }

            for pair in range(N_PAIRS):
                bg = slot_bg[pair]
                for g in range(NG):
                    if pair == 2 and g >= 1:
                        # proj tiles for the previous group are now valid
                        bg = bg + [lambda t=t: emit_proj_tile(t)
                                   for t in range(4 * (g - 1), 4 * g)]
                    # pull a fair share of bg into this group's slack
                    njt = 4 * g + 4
                    rem_j = sum(4 * gg + 4 for gg in range(g, NG))
                    take = (len(bg) * njt + rem_j - 1) // rem_j if rem_j else len(bg)
                    if pair == 2 and g >= 1:
                        take = len(bg) - (12 - 4 * g)  # keep only not-yet-valid proj
                    mine, bg = bg[:take], bg[take:]
                    emit_attn_group(pair, g, mine)
                slot_bg[pair] = bg

            # drain: anything not pulled + final proj tiles
            for u in slot_bg[2]:
                u()
            for t in range(12, 16):
                emit_proj_tile(t)

    nc.compile()
    return nc


def _numpy_fallback(x, mask, W_attn, b_attn, W_proj, b_proj):
    qkv = x @ W_attn + b_attn
    q, k, v = np.split(qkv, 3, axis=-1)

    def heads(t):
        return t.reshape(B, S, N_HEAD, HEAD_DIM).transpose(0, 2, 1, 3)

    q, k, v = heads(q), heads(k), heads(v)
    attn = np.einsum("bhqd,bhkd->bhqk", q, k) / np.sqrt(np.float32(HEAD_DIM))
    attn = attn + mask * (-1e9)
    attn = attn - attn.max(axis=-1, keepdims=True)
    attn = np.exp(attn)
    attn = attn / attn.sum(axis=-1, keepdims=True)
    out = np.einsum("bhqk,bhkd->bhqd", attn, v)
    out = out.transpose(0, 2, 1, 3).reshape(B, S, N_EMBD)
    return (out @ W_proj + b_proj).astype(np.float32)


def make_in_maps(x, W_attn, b_attn, W_proj):
    bf16 = ml_dtypes.bfloat16
    in_maps = []
    for c in range(N_CORES):
        b, hg = divmod(c, 2)
        o = HG_DIM * hg
        in_maps.append({
            "xT": np.ascontiguousarray(x[b].T.astype(bf16)),
            "w_qkv": np.ascontiguousarray(np.concatenate(
                [W_attn[:, o:o + HG_DIM],
                 W_attn[:, 768 + o:768 + o + HG_DIM],
                 W_attn[:, 1536 + o:1536 + o + HG_DIM]], axis=1).astype(bf16)),
            "b_qk": np.ascontiguousarray(np.concatenate(
                [b_attn[o:o + HG_DIM], b_attn[768 + o:768 + o + HG_DIM]])),
            "b_v": np.ascontiguousarray(b_attn[1536 + o:1536 + o + HG_DIM]).astype(bf16),
            "w_proj": np.ascontiguousarray(W_proj[o:o + HG_DIM, :].astype(bf16)),
            "ones": np.ones((1, 128), dtype=bf16),
        })
    return in_maps


def kernel(x, mask, W_attn, b_attn, W_proj, b_proj):
    global LAST_RESULTS, _PROGRAM
    x = np.asarray(x, dtype=np.float32)
    mask = np.asarray(mask, dtype=np.float32)
    W_attn = np.asarray(W_attn, dtype=np.float32)
    b_attn = np.asarray(b_attn, dtype=np.float32)
    W_proj = np.asarray(W_proj, dtype=np.float32)
    b_proj = np.asarray(b_proj, dtype=np.float32)

    # the kernel exploits causal structure; verify the mask actually is causal
    causal = 1.0 - np.tril(np.ones((S, S), dtype=np.float32))
    if mask.shape != (1, 1, S, S) or not np.array_equal(mask[0, 0], causal):
        return _numpy_fallback(x, mask, W_attn, b_attn, W_proj, b_proj)

    from concourse.bass_utils import run_bass_kernel_spmd

    if _PROGRAM is None:
        _PROGRAM = _build_program()

    in_maps = make_in_maps(x, W_attn, b_attn, W_proj)

    trace = bool(int(os.environ.get("ATTN_KERNEL_TRACE", "0")))
    res = run_bass_kernel_spmd(_PROGRAM, in_maps, list(range(N_CORES)), trace=trace)
    LAST_RESULTS = res

    y = np.zeros((B, S, N_EMBD), dtype=np.float32)
    for c in range(N_CORES):
        y[c // 2] += res.results[c]["y"]
    y += b_proj
    return y


# revision 5
# speedup vs baseline: 1.3687x; 1.0356x over previous
"""Trainium2 Bass kernel for a 12-head causal attention block (GPT-2 style).

Problem: x:[4,2048,768] -> qkv = x@W_attn+b_attn, causal softmax attention
(12 heads, d=64), out @ W_proj + b_proj.

Sharding over 8 NeuronCores: core c handles batch b=c//2 (data parallel) and
head-group hg=c%2 (6 heads = 3 head-pairs, tensor parallel on the qkv
columns / proj rows).  Each core returns a partial projection output; the
host sums the two head-group partials per batch and adds b_proj.

v2 design (vs the 410us baseline):
  - x is transposed and bf16-cast on the HOST: no PE-transpose phase, half
    the input DMA bytes, and every matmul operand is bf16 (fast weight
    load applies; PSUM accumulation stays fp32).
  - scores: the two heads of a pair run CONCURRENTLY in the PE array via
    row tiling (tile_position (0,0)/(64,0), K=64 each) - halves score time.
  - per-(pair,g) attention group: j-loop over k-tiles software-pipelined
    one stage deep (scores j+1 emitted before AV j) so the ACT exp of tile
    j overlaps the scores matmul of j+1.
  - AV uses the M=65 ones-column trick: attention output AND softmax
    denominators from one accumulating matmul per head.
  - normalization: DVE reciprocal -> GPSIMD partition_broadcast -> DVE
    multiply (no PE broadcast matmul, no PSUM->SBUF bounce of it).
  - qkv / proj matmuls are emitted as small work units INTERLEAVED into the
    attention j-loops: the PE executes them while ACT (the per-group
    bottleneck at ~1 elem/cycle/lane) chews on exp, keeping the PE dense so
    the HAM clock stays at 2.4 GHz.
  - PSUM budget: scores 2x[128,1024] (4 banks) + AV 2x[65,512] (2 banks) +
    shared aux pool 2x[128,512] (2 banks) = 8 banks exactly.
"""

import os
import ml_dtypes
import numpy as np

N_HEAD = 12
N_EMBD = 768
HEAD_DIM = 64
B, S = 4, 2048
N_CORES = 8
HG_HEADS = 6            # heads per core (3 pairs)
HG_DIM = HG_HEADS * HEAD_DIM   # 384
QKV_W = 3 * HG_DIM      # 1152 qkv columns per core
N_PAIRS = 3
ST = S // 128           # 16 seq tiles of 128
NG = S // 512           # 4 seq groups of 512

LAST_RESULTS = None
_PROGRAM = None


def _build_program():
    import concourse.bacc as bacc
    import concourse.tile as tile
    from concourse import mybir

    F32 = mybir.dt.float32
    BF16 = mybir.dt.bfloat16
    AF = mybir.ActivationFunctionType

    nc = bacc.Bacc(None, target_bir_lowering=False)
    xT_d = nc.declare_dram_parameter("xT", [N_EMBD, S], BF16, isOutput=False)
    wqkv_d = nc.declare_dram_parameter("w_qkv", [N_EMBD, QKV_W], BF16, isOutput=False)
    bqk_d = nc.declare_dram_parameter("b_qk", [768], F32, isOutput=False)
    bv_d = nc.declare_dram_parameter("b_v", [HG_DIM], BF16, isOutput=False)
    wproj_d = nc.declare_dram_parameter("w_proj", [HG_DIM, N_EMBD], BF16, isOutput=False)
    ones_d = nc.declare_dram_parameter("ones", [1, 128], BF16, isOutput=False)
    y_d = nc.declare_dram_parameter("y", [S, N_EMBD], F32, isOutput=True)

    with tile.TileContext(nc) as tc:
        from contextlib import ExitStack

        with ExitStack() as outer:
            consts = outer.enter_context(tc.tile_pool(name="consts", bufs=1))
            ones_row = consts.tile([1, 128], BF16)
            nc.sync.dma_start(out=ones_row[:], in_=ones_d[:])
            bias_qk = consts.tile([128, 6], F32)      # col m: b_qk[128m:128m+128]
            nc.sync.dma_start(
                out=bias_qk[:], in_=bqk_d[0:768].rearrange("(m p) -> p m", p=128)
            )
            bias_v = consts.tile([1, HG_DIM], BF16)
            nc.sync.dma_start(
                out=bias_v[:], in_=bv_d[0:HG_DIM].rearrange("(o v) -> o v", o=1)
            )

            # ---- persistent activations/weights in SBUF (all bf16) ----
            big = outer.enter_context(tc.tile_pool(name="big", bufs=1))
            xT = big.tile([128, 6 * S], BF16)       # [emb-part, k-chunk*2048+seq]
            w_all = big.tile([128, 6 * QKV_W], BF16)
            w_proj = big.tile([128, N_PAIRS * N_EMBD], BF16)
            qkT = big.tile([128, 6 * S], BF16)      # m=0..2 qT pairs, m=3..5 kT pairs
            # per k-tile: 6 heads x (64 v-cols + a ones col for the softmax
            # denominator) -> P@V and row-sums come from one M=65 matmul
            v_all = big.tile([128, ST * 390], BF16)  # [seq, t*390 + 65h + d]
            attnT = big.tile([128, N_PAIRS * S], BF16)

            nc.gpsimd.memset(v_all[:], 1.0)
            for k in range(6):
                nc.sync.dma_start(out=xT[:, k * S:(k + 1) * S],
                                  in_=xT_d[k * 128:(k + 1) * 128, :])
                nc.sync.dma_start(out=w_all[:, k * QKV_W:(k + 1) * QKV_W],
                                  in_=wqkv_d[k * 128:(k + 1) * 128, :])
            for p in range(N_PAIRS):
                nc.sync.dma_start(out=w_proj[:, p * N_EMBD:(p + 1) * N_EMBD],
                                  in_=wproj_d[p * 128:(p + 1) * 128, :])

            # ---- pools ----
            stps = outer.enter_context(tc.tile_pool(name="stps", bufs=2, space="PSUM"))
            avps = outer.enter_context(tc.tile_pool(name="avps", bufs=2, space="PSUM"))
            auxps = outer.enter_context(tc.tile_pool(name="auxps", bufs=2, space="PSUM"))
            ptp = outer.enter_context(tc.tile_pool(name="ptp", bufs=3))
            avsb = outer.enter_context(tc.tile_pool(name="avsb", bufs=4))
            rcp = outer.enter_context(tc.tile_pool(name="rcp", bufs=4))
            bcp = outer.enter_context(tc.tile_pool(name="bcp", bufs=4))
            shtmp = outer.enter_context(tc.tile_pool(name="shtmp", bufs=2))
            ystage = outer.enter_context(tc.tile_pool(name="ystage", bufs=3))

            v_view = v_all[:].rearrange("p (t h c) -> p t h c", t=ST, h=HG_HEADS)

            # ---- work-unit emitters (each emits a small PE-dense chunk) ----
            def emit_qk_group(m, g):
                # qkT[:, m*S + g*512 : +512] = (W[:, m-block].T @ xT)[:, g-block] + bias
                ps = auxps.tile([128, 512], F32, tag="aux")
                for k in range(6):
                    nc.tensor.matmul(
                        ps[:],
                        w_all[:, k * QKV_W + m * 128:k * QKV_W + (m + 1) * 128],
                        xT[:, k * S + g * 512:k * S + (g + 1) * 512],
                        start=(k == 0), stop=(k == 5),
                    )
                nc.vector.tensor_scalar_add(
                    qkT[:, m * S + g * 512:m * S + (g + 1) * 512],
                    ps[:], bias_qk[:, m:m + 1],
                )

            def emit_v_tile(pair, t):
                # v rows t*128.. for this pair's two heads (N=128)
                ps = auxps.tile([128, 128], F32, tag="aux")
                wc0 = 768 + pair * 128
                for k in range(6):
                    nc.tensor.matmul(
                        ps[:],
                        xT[:, k * S + t * 128:k * S + (t + 1) * 128],
                        w_all[:, k * QKV_W + wc0:k * QKV_W + wc0 + 128],
                        start=(k == 0), stop=False,
                    )
                nc.tensor.matmul(   # += ones^T[1,128].T @ bias_v[1,128]
                    ps[:], ones_row[:], bias_v[:, pair * 128:(pair + 1) * 128],
                    start=False, stop=True,
                )
                nc.vector.tensor_copy(
                    v_view[:, t, 2 * pair:2 * pair + 2, 0:64],
                    ps[:].rearrange("p (h d) -> p h d", h=2),
                )

            def emit_proj_tile(t):
                psA = auxps.tile([128, 512], F32, tag="aux")
                psB = auxps.tile([128, 256], F32, tag="aux")
                for p in range(N_PAIRS):
                    lhsT = attnT[:, p * S + t * 128:p * S + (t + 1) * 128]
                    nc.tensor.matmul(psA[:], lhsT, w_proj[:, p * N_EMBD:p * N_EMBD + 512],
                                     start=(p == 0), stop=(p == N_PAIRS - 1))
                    nc.tensor.matmul(psB[:], lhsT,
                                     w_proj[:, p * N_EMBD + 512:(p + 1) * N_EMBD],
                                     start=(p == 0), stop=(p == N_PAIRS - 1))
                ys = ystage.tile([128, N_EMBD], F32)
                nc.vector.tensor_copy(ys[:, 0:512], psA[:])
                nc.vector.tensor_copy(ys[:, 512:768], psB[:])
                nc.sync.dma_start(out=y_d[t * 128:(t + 1) * 128, :], in_=ys[:])

            # ---- attention group with interleaved background units ----
            def emit_attn_group(pair, g, bg_units):
                """bg_units: list of 0-arg emitters pulled into PE slack."""
                q0 = pair * S
                k0 = (3 + pair) * S
                njt = 4 * g + 4
                av0 = avps.tile([65, 512], F32, tag="av")
                av1 = avps.tile([65, 512], F32, tag="av")
                sts = {}
                pts = {}

                def scores(j):
                    diag_r = j - 4 * g
                    c0 = 128 * diag_r if diag_r >= 0 else 0
                    st = stps.tile([128, 1024], F32, tag="st")
                    nc.tensor.matmul(
                        st[:, c0:512],
                        qkT[0:64, k0 + j * 128:k0 + (j + 1) * 128],
                        qkT[0:64, q0 + g * 512 + c0:q0 + (g + 1) * 512],
                        start=True, stop=True, tile_position=(0, 0),
                    )
                    nc.tensor.matmul(
                        st[:, 512 + c0:1024],
                        qkT[64:128, k0 + j * 128:k0 + (j + 1) * 128],
                        qkT[64:128, q0 + g * 512 + c0:q0 + (g + 1) * 512],
                        start=True, stop=True, tile_position=(64, 0),
                    )
                    sts[j] = (st, c0)

                def expmask(j):
                    st, c0 = sts.pop(j)
                    pt = ptp.tile([128, 1024], BF16, tag="pt")
                    nc.scalar.activation(pt[:, c0:1024], st[:, c0:1024],
                                         AF.Exp, bias=0.0, scale=0.125)
                    diag_r = j - 4 * g
                    if diag_r >= 0:
                        for h in range(2):
                            nc.gpsimd.affine_select(
                                out=pt[:, h * 512 + c0:h * 512 + c0 + 128],
                                in_=pt[:, h * 512 + c0:h * 512 + c0 + 128],
                                compare_op=mybir.AluOpType.is_ge,
                                fill=0.0, base=0,
                                pattern=[[1, 128]], channel_multiplier=-1,
                            )
                    pts[j] = (pt, c0)

                def av(j):
                    pt, c0 = pts.pop(j)
                    first, last = (j == 0), (j == njt - 1)
                    for h, avt in ((0, av0), (1, av1)):
                        nc.tensor.matmul(
                            avt[0:65, c0:512],
                            v_all[:, j * 390 + (2 * pair + h) * 65:
                                  j * 390 + (2 * pair + h) * 65 + 65],
                            pt[:, h * 512 + c0:(h + 1) * 512],
                            start=first, stop=last,
                        )

                # schedule bg unit pulls evenly across the j loop
                pulls = [0] * njt
                for i in range(len(bg_units)):
                    pulls[(i * njt) // len(bg_units)] += 1

                scores(0)
                expmask(0)
                for j in range(njt):
                    if j + 1 < njt:
                        scores(j + 1)
                        expmask(j + 1)
                    for _ in range(pulls[j]):
                        bg_units.pop(0)()
                    av(j)

                # evacuate the AV accumulators to SBUF with one fast copy per
                # head (frees the PSUM banks for the next group's AV almost
                # immediately); the recip/broadcast/multiply chain is DEFERRED
                # into the next group's instruction stream so it never stalls
                # the PE at the group boundary.
                avsb0 = avsb.tile([65, 512], F32, tag="avsb")
                avsb1 = avsb.tile([65, 512], F32, tag="avsb")
                nc.vector.tensor_copy(avsb0[:], av0[:])
                nc.vector.tensor_copy(avsb1[:], av1[:])

                def normalize():
                    cols = slice(pair * S + g * 512, pair * S + (g + 1) * 512)
                    with nc.allow_low_precision(reason="softmax normalize bf16"):
                        for h, avt in ((0, avsb0), (1, avsb1)):
                            rc = rcp.tile([1, 512], F32)
                            nc.vector.reciprocal(rc[:], avt[64:65, :])
                            bc = bcp.tile([64, 512], F32)
                            nc.gpsimd.partition_broadcast(bc[:], rc[:], channels=64)
                            if h == 0:
                                nc.vector.tensor_mul(attnT[0:64, cols],
                                                     avt[0:64, :], bc[:])
                            else:
                                # DVE lanes are partition-locked: odd head's
                                # rows 64-127 via an SBUF bounce + DMA shift
                                tmp = shtmp.tile([64, 512], BF16)
                                nc.vector.tensor_mul(tmp[:], avt[0:64, :], bc[:])
                                nc.sync.dma_start(out=attnT[64:128, cols],
                                                  in_=tmp[:])
                return normalize

            # ================= schedule =================
            # upfront: just enough qkv for attn(0, g0)
            emit_qk_group(3, 0)          # kT pair 0, seq 0-511
            emit_qk_group(0, 0)          # qT pair 0, seq 0-511
            for t in range(4):
                emit_v_tile(0, t)

            def pair_bg(p):
                """qkv units for pair p beyond (g0 kT/qT, v t0-3), in need order."""
                units = []
                for g in range(1, NG):
                    units.append(lambda m=3 + p, g=g: emit_qk_group(m, g))
                    units.append(lambda m=p, g=g: emit_qk_group(m, g))
                    units.extend(lambda pair=p, t=t: emit_v_tile(pair, t)
                                 for t in range(4 * g, 4 * g + 4))
                return units

            def pair_head(p):
                """the upfront units of pair p (needed before attn(p, g0))."""
                units = [lambda m=3 + p: emit_qk_group(m, 0),
                         lambda m=p: emit_qk_group(m, 0)]
                units.extend(lambda pair=p, t=t: emit_v_tile(pair, t)
                             for t in range(4))
                return units

            # background queues per attention slot
            slot_bg = {
                0: pair_bg(0) + pair_head(1) + pair_bg(1),
                1: pair_head(2) + pair_bg(2),
                2: [],   # proj units are appended per-group below
            }

            # pair-2 groups run DESCENDING so the final (short) g=0 group's
            # proj tiles are the only tail work
            group_order = {0: list(range(NG)), 1: list(range(NG)),
                           2: [3, 2, 1, 0]}

            deferred_norm = None
            for pair in range(N_PAIRS):
                bg = slot_bg[pair]
                done_groups = []
                for gi, g in enumerate(group_order[pair]):
                    if pair == 2 and done_groups:
                        # proj tiles for already-normalized groups are valid
                        gprev = done_groups[-1]
                        bg = bg + [lambda t=t: emit_proj_tile(t)
                                   for t in range(4 * gprev, 4 * gprev + 4)]
                    njt = 4 * g + 4
                    rem_j = sum(4 * gg + 4 for gg in group_order[pair][gi:])
                    take = (len(bg) * njt + rem_j - 1) // rem_j if rem_j else len(bg)
                    mine, bg = bg[:take], bg[take:]
                    if deferred_norm is not None:
                        mine = [deferred_norm] + mine
                    deferred_norm = emit_attn_group(pair, g, mine)
                    done_groups.append(g)
                slot_bg[pair] = bg

            # drain: last group's normalize + anything not pulled + tail proj
            if deferred_norm is not None:
                deferred_norm()
            for u in slot_bg[2]:
                u()
            for t in range(0, 4):
                emit_proj_tile(t)

    nc.compile()
    return nc


def _numpy_fallback(x, mask, W_attn, b_attn, W_proj, b_proj):
    qkv = x @ W_attn + b_attn
    q, k, v = np.split(qkv, 3, axis=-1)

    def heads(t):
        return t.reshape(B, S, N_HEAD, HEAD_DIM).transpose(0, 2, 1, 3)

    q, k, v = heads(q), heads(k), heads(v)
    attn = np.einsum("bhqd,bhkd->bhqk", q, k) / np.sqrt(np.float32(HEAD_DIM))
    attn = attn + mask * (-1e9)
    attn = attn - attn.max(axis=-1, keepdims=True)
    attn = np.exp(attn)
    attn = attn / attn.sum(axis=-1, keepdims=True)
    out = np.einsum("bhqk,bhkd->bhqd", attn, v)
    out = out.transpose(0, 2, 1, 3).reshape(B, S, N_EMBD)
    return (out @ W_proj + b_proj).astype(np.float32)


def make_in_maps(x, W_attn, b_attn, W_proj):
    bf16 = ml_dtypes.bfloat16
    in_maps = []
    for c in range(N_CORES):
        b, hg = divmod(c, 2)
        o = HG_DIM * hg
        in_maps.append({
            "xT": np.ascontiguousarray(x[b].T.astype(bf16)),
            "w_qkv": np.ascontiguousarray(np.concatenate(
                [W_attn[:, o:o + HG_DIM],
                 W_attn[:, 768 + o:768 + o + HG_DIM],
                 W_attn[:, 1536 + o:1536 + o + HG_DIM]], axis=1).astype(bf16)),
            "b_qk": np.ascontiguousarray(np.concatenate(
                [b_attn[o:o + HG_DIM], b_attn[768 + o:768 + o + HG_DIM]])),
            "b_v": np.ascontiguousarray(b_attn[1536 + o:1536 + o + HG_DIM]).astype(bf16),
            "w_proj": np.ascontiguousarray(W_proj[o:o + HG_DIM, :].astype(bf16)),
            "ones": np.ones((1, 128), dtype=bf16),
        })
    return in_maps


def kernel(x, mask, W_attn, b_attn, W_proj, b_proj):
    global LAST_RESULTS, _PROGRAM
    x = np.asarray(x, dtype=np.float32)
    mask = np.asarray(mask, dtype=np.float32)
    W_attn = np.asarray(W_attn, dtype=np.float32)
    b_attn = np.asarray(b_attn, dtype=np.float32)
    W_proj = np.asarray(W_proj, dtype=np.float32)
    b_proj = np.asarray(b_proj, dtype=np.float32)

    # the kernel exploits causal structure; verify the mask actually is causal
    causal = 1.0 - np.tril(np.ones((S, S), dtype=np.float32))
    if mask.shape != (1, 1, S, S) or not np.array_equal(mask[0, 0], causal):
        return _numpy_fallback(x, mask, W_attn, b_attn, W_proj, b_proj)

    from concourse.bass_utils import run_bass_kernel_spmd

    if _PROGRAM is None:
        _PROGRAM = _build_program()

    in_maps = make_in_maps(x, W_attn, b_attn, W_proj)

    trace = bool(int(os.environ.get("ATTN_KERNEL_TRACE", "0")))
    res = run_bass_kernel_spmd(_PROGRAM, in_maps, list(range(N_CORES)), trace=trace)
    LAST_RESULTS = res

    y = np.zeros((B, S, N_EMBD), dtype=np.float32)
    for c in range(N_CORES):
        y[c // 2] += res.results[c]["y"]
    y += b_proj
    return y


# revision 13
# speedup vs baseline: 1.4207x; 1.0380x over previous
"""Trainium2 Bass kernel for a 12-head causal attention block (GPT-2 style).

Problem: x:[4,2048,768] -> qkv = x@W_attn+b_attn, causal softmax attention
(12 heads, d=64), out @ W_proj + b_proj.

Sharding over 8 NeuronCores: core c handles batch b=c//2 (data parallel) and
head-group hg=c%2 (6 heads = 3 head-pairs, tensor parallel on the qkv
columns / proj rows).  Each core returns a partial projection output; the
host sums the two head-group partials per batch and adds b_proj.

v2 design (vs the 410us baseline):
  - x is transposed and bf16-cast on the HOST: no PE-transpose phase, half
    the input DMA bytes, and every matmul operand is bf16 (fast weight
    load applies; PSUM accumulation stays fp32).
  - scores: the two heads of a pair run CONCURRENTLY in the PE array via
    row tiling (tile_position (0,0)/(64,0), K=64 each) - halves score time.
  - per-(pair,g) attention group: j-loop over k-tiles software-pipelined
    one stage deep (scores j+1 emitted before AV j) so the ACT exp of tile
    j overlaps the scores matmul of j+1.
  - AV uses the M=65 ones-column trick: attention output AND softmax
    denominators from one accumulating matmul per head.
  - normalization: DVE reciprocal -> GPSIMD partition_broadcast -> DVE
    multiply (no PE broadcast matmul, no PSUM->SBUF bounce of it).
  - qkv / proj matmuls are emitted as small work units INTERLEAVED into the
    attention j-loops: the PE executes them while ACT (the per-group
    bottleneck at ~1 elem/cycle/lane) chews on exp, keeping the PE dense so
    the HAM clock stays at 2.4 GHz.
  - PSUM budget: scores 2x[128,1024] (4 banks) + AV 2x[65,512] (2 banks) +
    shared aux pool 2x[128,512] (2 banks) = 8 banks exactly.
"""

import os
import ml_dtypes
import numpy as np

N_HEAD = 12
N_EMBD = 768
HEAD_DIM = 64
B, S = 4, 2048
N_CORES = 8
HG_HEADS = 6            # heads per core (3 pairs)
HG_DIM = HG_HEADS * HEAD_DIM   # 384
QKV_W = 3 * HG_DIM      # 1152 qkv columns per core
N_PAIRS = 3
ST = S // 128           # 16 seq tiles of 128
NG = S // 512           # 4 seq groups of 512

LAST_RESULTS = None
_PROGRAM = None


def _build_program():
    import concourse.bacc as bacc
    import concourse.tile as tile
    from concourse import mybir

    F32 = mybir.dt.float32
    BF16 = mybir.dt.bfloat16
    AF = mybir.ActivationFunctionType

    nc = bacc.Bacc(None, target_bir_lowering=False)
    xT_d = nc.declare_dram_parameter("xT", [N_EMBD, S], BF16, isOutput=False)
    wqkv_d = nc.declare_dram_parameter("w_qkv", [N_EMBD, QKV_W], BF16, isOutput=False)
    bqk_d = nc.declare_dram_parameter("b_qk", [768], F32, isOutput=False)
    bv_d = nc.declare_dram_parameter("b_v", [HG_DIM], BF16, isOutput=False)
    wproj_d = nc.declare_dram_parameter("w_proj", [HG_DIM, N_EMBD], BF16, isOutput=False)
    ones_d = nc.declare_dram_parameter("ones", [1, 128], BF16, isOutput=False)
    y_d = nc.declare_dram_parameter("y", [S, N_EMBD], F32, isOutput=True)

    with tile.TileContext(nc) as tc:
        from contextlib import ExitStack

        with ExitStack() as outer:
            consts = outer.enter_context(tc.tile_pool(name="consts", bufs=1))
            ones_row = consts.tile([1, 128], BF16)
            nc.sync.dma_start(out=ones_row[:], in_=ones_d[:])
            bias_qk = consts.tile([128, 6], F32)      # col m: b_qk[128m:128m+128]
            nc.sync.dma_start(
                out=bias_qk[:], in_=bqk_d[0:768].rearrange("(m p) -> p m", p=128)
            )
            bias_v = consts.tile([1, HG_DIM], BF16)
            nc.sync.dma_start(
                out=bias_v[:], in_=bv_d[0:HG_DIM].rearrange("(o v) -> o v", o=1)
            )

            # ---- persistent activations/weights in SBUF (all bf16) ----
            big = outer.enter_context(tc.tile_pool(name="big", bufs=1))
            xT = big.tile([128, 6 * S], BF16)       # [emb-part, k-chunk*2048+seq]
            w_all = big.tile([128, 6 * QKV_W], BF16)
            w_proj = big.tile([128, N_PAIRS * N_EMBD], BF16)
            qkT = big.tile([128, 6 * S], BF16)      # m=0..2 qT pairs, m=3..5 kT pairs
            # per k-tile: 6 heads x (64 v-cols + a ones col for the softmax
            # denominator) -> P@V and row-sums come from one M=65 matmul
            v_all = big.tile([128, ST * 390], BF16)  # [seq, t*390 + 65h + d]
            attnT = big.tile([128, N_PAIRS * S], BF16)

            nc.gpsimd.memset(v_all[:], 1.0)
            for k in range(6):
                nc.sync.dma_start(out=xT[:, k * S:(k + 1) * S],
                                  in_=xT_d[k * 128:(k + 1) * 128, :])
                nc.sync.dma_start(out=w_all[:, k * QKV_W:(k + 1) * QKV_W],
                                  in_=wqkv_d[k * 128:(k + 1) * 128, :])
            for p in range(N_PAIRS):
                nc.sync.dma_start(out=w_proj[:, p * N_EMBD:(p + 1) * N_EMBD],
                                  in_=wproj_d[p * 128:(p + 1) * 128, :])

            # ---- pools ----
            stps = outer.enter_context(tc.tile_pool(name="stps", bufs=2, space="PSUM"))
            avps = outer.enter_context(tc.tile_pool(name="avps", bufs=2, space="PSUM"))
            auxps = outer.enter_context(tc.tile_pool(name="auxps", bufs=2, space="PSUM"))
            ptp = outer.enter_context(tc.tile_pool(name="ptp", bufs=3))
            avsb = outer.enter_context(tc.tile_pool(name="avsb", bufs=4))
            rcp = outer.enter_context(tc.tile_pool(name="rcp", bufs=4))
            bcp = outer.enter_context(tc.tile_pool(name="bcp", bufs=4))
            shtmp = outer.enter_context(tc.tile_pool(name="shtmp", bufs=2))
            ystage = outer.enter_context(tc.tile_pool(name="ystage", bufs=3))

            v_view = v_all[:].rearrange("p (t h c) -> p t h c", t=ST, h=HG_HEADS)

            # ---- work-unit emitters (each emits a small PE-dense chunk) ----
            def emit_qk_group(m, g):
                # qkT[:, m*S + g*512 : +512] = (W[:, m-block].T @ xT)[:, g-block] + bias
                ps = auxps.tile([128, 512], F32, tag="aux")
                for k in range(6):
                    nc.tensor.matmul(
                        ps[:],
                        w_all[:, k * QKV_W + m * 128:k * QKV_W + (m + 1) * 128],
                        xT[:, k * S + g * 512:k * S + (g + 1) * 512],
                        start=(k == 0), stop=(k == 5),
                    )
                nc.vector.tensor_scalar_add(
                    qkT[:, m * S + g * 512:m * S + (g + 1) * 512],
                    ps[:], bias_qk[:, m:m + 1],
                )

            def emit_v_tile(pair, t):
                # v rows t*128.. for this pair's two heads (N=128)
                ps = auxps.tile([128, 128], F32, tag="aux")
                wc0 = 768 + pair * 128
                for k in range(6):
                    nc.tensor.matmul(
                        ps[:],
                        xT[:, k * S + t * 128:k * S + (t + 1) * 128],
                        w_all[:, k * QKV_W + wc0:k * QKV_W + wc0 + 128],
                        start=(k == 0), stop=False,
                    )
                nc.tensor.matmul(   # += ones^T[1,128].T @ bias_v[1,128]
                    ps[:], ones_row[:], bias_v[:, pair * 128:(pair + 1) * 128],
                    start=False, stop=True,
                )
                nc.vector.tensor_copy(
                    v_view[:, t, 2 * pair:2 * pair + 2, 0:64],
                    ps[:].rearrange("p (h d) -> p h d", h=2),
                )

            def emit_proj_tile(t):
                psA = auxps.tile([128, 512], F32, tag="aux")
                psB = auxps.tile([128, 256], F32, tag="aux")
                for p in range(N_PAIRS):
                    lhsT = attnT[:, p * S + t * 128:p * S + (t + 1) * 128]
                    nc.tensor.matmul(psA[:], lhsT, w_proj[:, p * N_EMBD:p * N_EMBD + 512],
                                     start=(p == 0), stop=(p == N_PAIRS - 1))
                    nc.tensor.matmul(psB[:], lhsT,
                                     w_proj[:, p * N_EMBD + 512:(p + 1) * N_EMBD],
                                     start=(p == 0), stop=(p == N_PAIRS - 1))
                ys = ystage.tile([128, N_EMBD], F32)
                nc.vector.tensor_copy(ys[:, 0:512], psA[:])
                nc.vector.tensor_copy(ys[:, 512:768], psB[:])
                nc.sync.dma_start(out=y_d[t * 128:(t + 1) * 128, :], in_=ys[:])

            # ---- attention group with interleaved background units ----
            def emit_attn_group(pair, g, bg_units, pre_unit=None):
                """bg_units: list of 0-arg emitters pulled into PE slack.
                pre_unit: emitted right after the pipeline warm-up, BEFORE any
                bg unit (the deferred normalize must precede proj units that
                read the attnT columns it writes)."""
                q0 = pair * S
                k0 = (3 + pair) * S
                njt = 4 * g + 4
                av0 = avps.tile([65, 512], F32, tag="av")
                av1 = avps.tile([65, 512], F32, tag="av")
                sts = {}
                pts = {}

                def scores(j):
                    diag_r = j - 4 * g
                    c0 = 128 * diag_r if diag_r >= 0 else 0
                    st = stps.tile([128, 1024], F32, tag="st")
                    nc.tensor.matmul(
                        st[:, c0:512],
                        qkT[0:64, k0 + j * 128:k0 + (j + 1) * 128],
                        qkT[0:64, q0 + g * 512 + c0:q0 + (g + 1) * 512],
                        start=True, stop=True, tile_position=(0, 0),
                    )
                    nc.tensor.matmul(
                        st[:, 512 + c0:1024],
                        qkT[64:128, k0 + j * 128:k0 + (j + 1) * 128],
                        qkT[64:128, q0 + g * 512 + c0:q0 + (g + 1) * 512],
                        start=True, stop=True, tile_position=(64, 0),
                    )
                    sts[j] = (st, c0)

                def expmask(j):
                    st, c0 = sts.pop(j)
                    pt = ptp.tile([128, 1024], BF16, tag="pt")
                    nc.scalar.activation(pt[:, c0:1024], st[:, c0:1024],
                                         AF.Exp, bias=0.0, scale=0.125)
                    diag_r = j - 4 * g
                    if diag_r >= 0:
                        for h in range(2):
                            nc.gpsimd.affine_select(
                                out=pt[:, h * 512 + c0:h * 512 + c0 + 128],
                                in_=pt[:, h * 512 + c0:h * 512 + c0 + 128],
                                compare_op=mybir.AluOpType.is_ge,
                                fill=0.0, base=0,
                                pattern=[[1, 128]], channel_multiplier=-1,
                            )
                    pts[j] = (pt, c0)

                def av(j):
                    pt, c0 = pts.pop(j)
                    first, last = (j == 0), (j == njt - 1)
                    for h, avt in ((0, av0), (1, av1)):
                        nc.tensor.matmul(
                            avt[0:65, c0:512],
                            v_all[:, j * 390 + (2 * pair + h) * 65:
                                  j * 390 + (2 * pair + h) * 65 + 65],
                            pt[:, h * 512 + c0:(h + 1) * 512],
                            start=first, stop=last,
                        )

                # schedule bg unit pulls evenly across the j loop
                pulls = [0] * njt
                for i in range(len(bg_units)):
                    pulls[(i * njt) // len(bg_units)] += 1

                scores(0)
                expmask(0)
                if pre_unit is not None:
                    pre_unit()
                for j in range(njt):
                    if j + 1 < njt:
                        scores(j + 1)
                        expmask(j + 1)
                    for _ in range(pulls[j]):
                        bg_units.pop(0)()
                    av(j)

                # evacuate the AV accumulators to SBUF with one fast copy per
                # head (frees the PSUM banks for the next group's AV almost
                # immediately); the recip/broadcast/multiply chain is DEFERRED
                # into the next group's instruction stream so it never stalls
                # the PE at the group boundary.
                avsb0 = avsb.tile([65, 512], F32, tag="avsb")
                avsb1 = avsb.tile([65, 512], F32, tag="avsb")
                nc.vector.tensor_copy(avsb0[:], av0[:])
                nc.vector.tensor_copy(avsb1[:], av1[:])

                def normalize():
                    cols = slice(pair * S + g * 512, pair * S + (g + 1) * 512)
                    # DVE reciprocal runs ~9 cyc/elem PER LANE: on [1,512] it
                    # costs 3.3us and head-of-line-blocks the DVE queue.
                    # Reshape both heads' denominators to [128,8] via SBUF
                    # DMAs (row-major pairing, probe-verified) so the recip
                    # uses 128 lanes (~0.1us), then shape back for the
                    # gpsimd partition broadcast.
                    dn8 = rcp.tile([128, 8], F32, tag="dn8")
                    nc.sync.dma_start(out=dn8[0:64, :], in_=avsb0[64:65, :])
                    nc.sync.dma_start(out=dn8[64:128, :], in_=avsb1[64:65, :])
                    rc8 = rcp.tile([128, 8], F32, tag="rc8")
                    with nc.allow_low_precision(reason="softmax normalize bf16"):
                        nc.vector.reciprocal(rc8[:], dn8[:])
                        for h, avt in ((0, avsb0), (1, avsb1)):
                            rc = rcp.tile([1, 512], F32, tag="rcrow")
                            nc.sync.dma_start(out=rc[:],
                                              in_=rc8[64 * h:64 * h + 64, :])
                            bc = bcp.tile([64, 512], F32)
                            nc.gpsimd.partition_broadcast(bc[:], rc[:],
                                                          channels=64)
                            if h == 0:
                                nc.vector.tensor_mul(attnT[0:64, cols],
                                                     avt[0:64, :], bc[:])
                            else:
                                # DVE lanes are partition-locked: odd head's
                                # rows 64-127 via an SBUF bounce + DMA shift
                                tmp = shtmp.tile([64, 512], BF16)
                                nc.vector.tensor_mul(tmp[:], avt[0:64, :], bc[:])
                                nc.sync.dma_start(out=attnT[64:128, cols],
                                                  in_=tmp[:])
                return normalize

            # ================= schedule =================
            # upfront: just enough qkv for attn(0, g0)
            emit_qk_group(3, 0)          # kT pair 0, seq 0-511
            emit_qk_group(0, 0)          # qT pair 0, seq 0-511
            for t in range(4):
                emit_v_tile(0, t)

            def pair_bg(p):
                """qkv units for pair p beyond (g0 kT/qT, v t0-3), in need order."""
                units = []
                for g in range(1, NG):
                    units.append(lambda m=3 + p, g=g: emit_qk_group(m, g))
                    units.append(lambda m=p, g=g: emit_qk_group(m, g))
                    units.extend(lambda pair=p, t=t: emit_v_tile(pair, t)
                                 for t in range(4 * g, 4 * g + 4))
                return units

            def pair_head(p):
                """the upfront units of pair p (needed before attn(p, g0))."""
                units = [lambda m=3 + p: emit_qk_group(m, 0),
                         lambda m=p: emit_qk_group(m, 0)]
                units.extend(lambda pair=p, t=t: emit_v_tile(pair, t)
                             for t in range(4))
                return units

            # background queues per attention slot
            slot_bg = {
                0: pair_bg(0) + pair_head(1) + pair_bg(1),
                1: pair_head(2) + pair_bg(2),
                2: [],   # proj units are appended per-group below
            }

            # pair-2 groups run DESCENDING so the final (short) g=0 group's
            # proj tiles are the only tail work
            group_order = {0: list(range(NG)), 1: list(range(NG)),
                           2: [3, 2, 1, 0]}

            deferred_norm = None
            for pair in range(N_PAIRS):
                bg = slot_bg[pair]
                done_groups = []
                for gi, g in enumerate(group_order[pair]):
                    if pair == 2 and done_groups:
                        # proj tiles for already-normalized groups are valid
                        gprev = done_groups[-1]
                        bg = bg + [lambda t=t: emit_proj_tile(t)
                                   for t in range(4 * gprev, 4 * gprev + 4)]
                    njt = 4 * g + 4
                    rem_j = sum(4 * gg + 4 for gg in group_order[pair][gi:])
                    take = (len(bg) * njt + rem_j - 1) // rem_j if rem_j else len(bg)
                    mine, bg = bg[:take], bg[take:]
                    deferred_norm = emit_attn_group(pair, g, mine,
                                                    pre_unit=deferred_norm)
                    done_groups.append(g)
                slot_bg[pair] = bg

            # drain: last group's normalize + anything not pulled + tail proj
            if deferred_norm is not None:
                deferred_norm()
            for u in slot_bg[2]:
                u()
            for t in range(0, 4):
                emit_proj_tile(t)

    nc.compile()
    return nc


def _numpy_fallback(x, mask, W_attn, b_attn, W_proj, b_proj):
    qkv = x @ W_attn + b_attn
    q, k, v = np.split(qkv, 3, axis=-1)

    def heads(t):
        return t.reshape(B, S, N_HEAD, HEAD_DIM).transpose(0, 2, 1, 3)

    q, k, v = heads(q), heads(k), heads(v)
    attn = np.einsum("bhqd,bhkd->bhqk", q, k) / np.sqrt(np.float32(HEAD_DIM))
    attn = attn + mask * (-1e9)
    attn = attn - attn.max(axis=-1, keepdims=True)
    attn = np.exp(attn)
    attn = attn / attn.sum(axis=-1, keepdims=True)
    out = np.einsum("bhqk,bhkd->bhqd", attn, v)
    out = out.transpose(0, 2, 1, 3).reshape(B, S, N_EMBD)
    return (out @ W_proj + b_proj).astype(np.float32)


def make_in_maps(x, W_attn, b_attn, W_proj):
    bf16 = ml_dtypes.bfloat16
    in_maps = []
    for c in range(N_CORES):
        b, hg = divmod(c, 2)
        o = HG_DIM * hg
        in_maps.append({
            "xT": np.ascontiguousarray(x[b].T.astype(bf16)),
            "w_qkv": np.ascontiguousarray(np.concatenate(
                [W_attn[:, o:o + HG_DIM],
                 W_attn[:, 768 + o:768 + o + HG_DIM],
                 W_attn[:, 1536 + o:1536 + o + HG_DIM]], axis=1).astype(bf16)),
            "b_qk": np.ascontiguousarray(np.concatenate(
                [b_attn[o:o + HG_DIM], b_attn[768 + o:768 + o + HG_DIM]])),
            "b_v": np.ascontiguousarray(b_attn[1536 + o:1536 + o + HG_DIM]).astype(bf16),
            "w_proj": np.ascontiguousarray(W_proj[o:o + HG_DIM, :].astype(bf16)),
            "ones": np.ones((1, 128), dtype=bf16),
        })
    return in_maps


def kernel(x, mask, W_attn, b_attn, W_proj, b_proj):
    global LAST_RESULTS, _PROGRAM
    x = np.asarray(x, dtype=np.float32)
    mask = np.asarray(mask, dtype=np.float32)
    W_attn = np.asarray(W_attn, dtype=np.float32)
    b_attn = np.asarray(b_attn, dtype=np.float32)
    W_proj = np.asarray(W_proj, dtype=np.float32)
    b_proj = np.asarray(b_proj, dtype=np.float32)

    # the kernel exploits causal structure; verify the mask actually is causal
    causal = 1.0 - np.tril(np.ones((S, S), dtype=np.float32))
    if mask.shape != (1, 1, S, S) or not np.array_equal(mask[0, 0], causal):
        return _numpy_fallback(x, mask, W_attn, b_attn, W_proj, b_proj)

    from concourse.bass_utils import run_bass_kernel_spmd

    if _PROGRAM is None:
        _PROGRAM = _build_program()

    in_maps = make_in_maps(x, W_attn, b_attn, W_proj)

    trace = bool(int(os.environ.get("ATTN_KERNEL_TRACE", "0")))
    res = run_bass_kernel_spmd(_PROGRAM, in_maps, list(range(N_CORES)), trace=trace)
    LAST_RESULTS = res

    y = np.zeros((B, S, N_EMBD), dtype=np.float32)
    for c in range(N_CORES):
        y[c // 2] += res.results[c]["y"]
    y += b_proj
    return y


# revision 23
# speedup vs baseline: 1.7284x; 1.2166x over previous
"""Trainium2 Bass kernel for a 12-head causal attention block (GPT-2 style).

Problem: x:[4,2048,768] -> qkv = x@W_attn+b_attn, causal softmax attention
(12 heads, d=64), out @ W_proj + b_proj.

Sharding over 8 NeuronCores: core c handles batch b=c//2 (data parallel) and
head-group hg=c%2 (6 heads = 3 head-pairs, tensor parallel on the qkv
columns / proj rows).  Each core returns a partial projection output; the
host sums the two head-group partials per batch and adds b_proj.

v2 design (vs the 410us baseline):
  - x is transposed and bf16-cast on the HOST: no PE-transpose phase, half
    the input DMA bytes, and every matmul operand is bf16 (fast weight
    load applies; PSUM accumulation stays fp32).
  - scores: the two heads of a pair run CONCURRENTLY in the PE array via
    row tiling (tile_position (0,0)/(64,0), K=64 each) - halves score time.
  - per-(pair,g) attention group: j-loop over k-tiles software-pipelined
    one stage deep (scores j+1 emitted before AV j) so the ACT exp of tile
    j overlaps the scores matmul of j+1.
  - AV uses the M=65 ones-column trick: attention output AND softmax
    denominators from one accumulating matmul per head.
  - normalization: DVE reciprocal -> GPSIMD partition_broadcast -> DVE
    multiply (no PE broadcast matmul, no PSUM->SBUF bounce of it).
  - qkv / proj matmuls are emitted as small work units INTERLEAVED into the
    attention j-loops: the PE executes them while ACT (the per-group
    bottleneck at ~1 elem/cycle/lane) chews on exp, keeping the PE dense so
    the HAM clock stays at 2.4 GHz.
  - PSUM budget: scores 2x[128,1024] (4 banks) + AV 2x[65,512] (2 banks) +
    shared aux pool 2x[128,512] (2 banks) = 8 banks exactly.
"""

import os
import ml_dtypes
import numpy as np

N_HEAD = 12
N_EMBD = 768
HEAD_DIM = 64
B, S = 4, 2048
N_CORES = 8
HG_HEADS = 6            # heads per core (3 pairs)
HG_DIM = HG_HEADS * HEAD_DIM   # 384
QKV_W = 3 * HG_DIM      # 1152 qkv columns per core
N_PAIRS = 3
ST = S // 128           # 16 seq tiles of 128
NG = S // 512           # 4 seq groups of 512

LAST_RESULTS = None
_PROGRAM = None


def _build_program():
    import concourse.bacc as bacc
    import concourse.tile as tile
    from concourse import mybir

    F32 = mybir.dt.float32
    BF16 = mybir.dt.bfloat16
    AF = mybir.ActivationFunctionType

    nc = bacc.Bacc(None, target_bir_lowering=False)
    xT_d = nc.declare_dram_parameter("xT", [N_EMBD, S], BF16, isOutput=False)
    wqkv_d = nc.declare_dram_parameter("w_qkv", [N_EMBD, QKV_W], BF16, isOutput=False)
    bqk_d = nc.declare_dram_parameter("b_qk", [768], F32, isOutput=False)
    bv_d = nc.declare_dram_parameter("b_v", [HG_DIM], BF16, isOutput=False)
    wproj_d = nc.declare_dram_parameter("w_proj", [HG_DIM, N_EMBD], BF16, isOutput=False)
    ones_d = nc.declare_dram_parameter("ones", [1, 128], BF16, isOutput=False)
    y_d = nc.declare_dram_parameter("y", [S, N_EMBD], F32, isOutput=True)

    with tile.TileContext(nc) as tc:
        from contextlib import ExitStack

        with ExitStack() as outer:
            consts = outer.enter_context(tc.tile_pool(name="consts", bufs=1))
            ones_row = consts.tile([1, 128], BF16)
            nc.sync.dma_start(out=ones_row[:], in_=ones_d[:])
            bias_qk = consts.tile([128, 6], F32)      # col m: b_qk[128m:128m+128]
            nc.sync.dma_start(
                out=bias_qk[:], in_=bqk_d[0:768].rearrange("(m p) -> p m", p=128)
            )
            bias_v = consts.tile([1, HG_DIM], BF16)
            nc.sync.dma_start(
                out=bias_v[:], in_=bv_d[0:HG_DIM].rearrange("(o v) -> o v", o=1)
            )

            # ---- persistent activations/weights in SBUF (all bf16) ----
            big = outer.enter_context(tc.tile_pool(name="big", bufs=1))
            xT = big.tile([128, 6 * S], BF16)       # [emb-part, k-chunk*2048+seq]
            w_all = big.tile([128, 6 * QKV_W], BF16)
            w_proj = big.tile([128, N_PAIRS * N_EMBD], BF16)
            qkT = big.tile([128, 6 * S], BF16)      # m=0..2 qT pairs, m=3..5 kT pairs
            # per k-tile: 6 heads x (64 v-cols + a ones col for the softmax
            # denominator) -> P@V and row-sums come from one M=65 matmul
            v_all = big.tile([128, ST * 390], BF16)  # [seq, t*390 + 65h + d]
            attnT = big.tile([128, N_PAIRS * S], BF16)

            nc.gpsimd.memset(v_all[:], 1.0)
            for k in range(6):
                nc.sync.dma_start(out=xT[:, k * S:(k + 1) * S],
                                  in_=xT_d[k * 128:(k + 1) * 128, :])
                nc.sync.dma_start(out=w_all[:, k * QKV_W:(k + 1) * QKV_W],
                                  in_=wqkv_d[k * 128:(k + 1) * 128, :])
            for p in range(N_PAIRS):
                nc.sync.dma_start(out=w_proj[:, p * N_EMBD:(p + 1) * N_EMBD],
                                  in_=wproj_d[p * 128:(p + 1) * 128, :])

            # ---- pools ----
            stps = outer.enter_context(tc.tile_pool(name="stps", bufs=2, space="PSUM"))
            avps = outer.enter_context(tc.tile_pool(name="avps", bufs=2, space="PSUM"))
            auxps = outer.enter_context(tc.tile_pool(name="auxps", bufs=2, space="PSUM"))
            ptp = outer.enter_context(tc.tile_pool(name="ptp", bufs=3))
            avsb = outer.enter_context(tc.tile_pool(name="avsb", bufs=4))
            rcp = outer.enter_context(tc.tile_pool(name="rcp", bufs=4))
            bcp = outer.enter_context(tc.tile_pool(name="bcp", bufs=4))
            shtmp = outer.enter_context(tc.tile_pool(name="shtmp", bufs=2))
            ystage = outer.enter_context(tc.tile_pool(name="ystage", bufs=3))

            v_view = v_all[:].rearrange("p (t h c) -> p t h c", t=ST, h=HG_HEADS)

            # ---- work-unit emitters (each emits a small PE-dense chunk) ----
            def emit_qk_group(m, g):
                # qkT[:, m*S + g*512 : +512] = (W[:, m-block].T @ xT)[:, g-block] + bias
                ps = auxps.tile([128, 512], F32, tag="aux")
                for k in range(6):
                    nc.tensor.matmul(
                        ps[:],
                        w_all[:, k * QKV_W + m * 128:k * QKV_W + (m + 1) * 128],
                        xT[:, k * S + g * 512:k * S + (g + 1) * 512],
                        start=(k == 0), stop=(k == 5),
                    )
                nc.vector.tensor_scalar_add(
                    qkT[:, m * S + g * 512:m * S + (g + 1) * 512],
                    ps[:], bias_qk[:, m:m + 1],
                )

            def emit_v_tile(t):
                # v rows t*128.. for all 6 heads (N=384)
                ps = auxps.tile([128, HG_DIM], F32, tag="aux")
                for k in range(6):
                    nc.tensor.matmul(
                        ps[:],
                        xT[:, k * S + t * 128:k * S + (t + 1) * 128],
                        w_all[:, k * QKV_W + 768:k * QKV_W + QKV_W],
                        start=(k == 0), stop=False,
                    )
                nc.tensor.matmul(   # += ones^T[1,128].T @ bias_v[1,384]
                    ps[:], ones_row[:], bias_v[:], start=False, stop=True,
                )
                nc.vector.tensor_copy(
                    v_view[:, t, :, 0:64],
                    ps[:].rearrange("p (h d) -> p h d", h=6),
                )

            def emit_proj_tile(t):
                psA = auxps.tile([128, 512], F32, tag="aux")
                psB = auxps.tile([128, 256], F32, tag="aux")
                for p in range(N_PAIRS):
                    lhsT = attnT[:, p * S + t * 128:p * S + (t + 1) * 128]
                    nc.tensor.matmul(psA[:], lhsT, w_proj[:, p * N_EMBD:p * N_EMBD + 512],
                                     start=(p == 0), stop=(p == N_PAIRS - 1))
                    nc.tensor.matmul(psB[:], lhsT,
                                     w_proj[:, p * N_EMBD + 512:(p + 1) * N_EMBD],
                                     start=(p == 0), stop=(p == N_PAIRS - 1))
                ys = ystage.tile([128, N_EMBD], F32)
                nc.vector.tensor_copy(ys[:, 0:512], psA[:])
                nc.vector.tensor_copy(ys[:, 512:768], psB[:])
                nc.sync.dma_start(out=y_d[t * 128:(t + 1) * 128, :], in_=ys[:])

            # ---- deadline-driven background work queue ----
            # Attention groups execute in a fixed order; (pair, g, j) maps to
            # a global step.  Each qkv/proj work unit carries the step by
            # which it MUST be emitted (Tile deps are emission-order-based:
            # a read emitted before its producer gets no dependency).  Units
            # are pulled with LOOKAHEAD steps of slack so the PE always has
            # background matmuls to chew on while ACT runs exp.
            group_order = {0: [0, 1, 2, 3], 1: [0, 1, 2, 3], 2: [3, 2, 1, 0]}
            step_base = {}
            _acc = 0
            for _p in range(N_PAIRS):
                for _g in group_order[_p]:
                    step_base[(_p, _g)] = _acc
                    _acc += 4 * _g + 4
            TOTAL_STEPS = _acc
            LOOKAHEAD = 9

            work_q = []   # sorted list of (deadline_step, seq, fn)
            _seq = [0]

            def push(deadline, fn):
                import bisect
                _seq[0] += 1
                bisect.insort(work_q, (deadline, _seq[0], fn))

            def pull_work(cur_step):
                # overdue units MUST emit now (correctness: emission order
                # defines Tile dependencies); otherwise spread at one unit
                # per step so the background work stays evenly interleaved.
                while work_q and work_q[0][0] <= cur_step:
                    work_q.pop(0)[2]()
                if work_q and work_q[0][0] <= cur_step + LOOKAHEAD:
                    work_q.pop(0)[2]()

            # ---- attention group with interleaved background units ----
            def emit_attn_group(pair, g, pre_unit=None):
                """pre_unit: emitted right after the pipeline warm-up, BEFORE
                any work-queue unit (the deferred normalize must precede proj
                units that read the attnT columns it writes)."""
                q0 = pair * S
                k0 = (3 + pair) * S
                njt = 4 * g + 4
                av0 = avps.tile([65, 512], F32, tag="av")
                av1 = avps.tile([65, 512], F32, tag="av")
                sts = {}
                pts = {}

                def scores(j):
                    diag_r = j - 4 * g
                    c0 = 128 * diag_r if diag_r >= 0 else 0
                    st = stps.tile([128, 1024], F32, tag="st")
                    nc.tensor.matmul(
                        st[:, c0:512],
                        qkT[0:64, k0 + j * 128:k0 + (j + 1) * 128],
                        qkT[0:64, q0 + g * 512 + c0:q0 + (g + 1) * 512],
                        start=True, stop=True, tile_position=(0, 0),
                    )
                    nc.tensor.matmul(
                        st[:, 512 + c0:1024],
                        qkT[64:128, k0 + j * 128:k0 + (j + 1) * 128],
                        qkT[64:128, q0 + g * 512 + c0:q0 + (g + 1) * 512],
                        start=True, stop=True, tile_position=(64, 0),
                    )
                    sts[j] = (st, c0)

                def expmask(j):
                    st, c0 = sts.pop(j)
                    pt = ptp.tile([128, 1024], BF16, tag="pt")
                    nc.scalar.activation(pt[:, c0:1024], st[:, c0:1024],
                                         AF.Exp, bias=0.0, scale=0.125)
                    diag_r = j - 4 * g
                    if diag_r >= 0:
                        for h in range(2):
                            nc.gpsimd.affine_select(
                                out=pt[:, h * 512 + c0:h * 512 + c0 + 128],
                                in_=pt[:, h * 512 + c0:h * 512 + c0 + 128],
                                compare_op=mybir.AluOpType.is_ge,
                                fill=0.0, base=0,
                                pattern=[[1, 128]], channel_multiplier=-1,
                            )
                    pts[j] = (pt, c0)

                def av(j):
                    pt, c0 = pts.pop(j)
                    first, last = (j == 0), (j == njt - 1)
                    for h, avt in ((0, av0), (1, av1)):
                        nc.tensor.matmul(
                            avt[0:65, c0:512],
                            v_all[:, j * 390 + (2 * pair + h) * 65:
                                  j * 390 + (2 * pair + h) * 65 + 65],
                            pt[:, h * 512 + c0:(h + 1) * 512],
                            start=first, stop=last,
                        )

                scores(0)
                expmask(0)
                if pre_unit is not None:
                    pre_unit()
                base = step_base[(pair, g)]
                for j in range(njt):
                    if j + 1 < njt:
                        scores(j + 1)
                        expmask(j + 1)
                    pull_work(base + j)
                    av(j)

                # evacuate the AV accumulators to SBUF with one fast copy per
                # head (frees the PSUM banks for the next group's AV almost
                # immediately); the recip/broadcast/multiply chain is DEFERRED
                # into the next group's instruction stream so it never stalls
                # the PE at the group boundary.
                avsb0 = avsb.tile([65, 512], F32, tag="avsb")
                avsb1 = avsb.tile([65, 512], F32, tag="avsb")
                nc.vector.tensor_copy(avsb0[:], av0[:])
                nc.vector.tensor_copy(avsb1[:], av1[:])

                def normalize():
                    cols = slice(pair * S + g * 512, pair * S + (g + 1) * 512)
                    # DVE reciprocal runs ~9 cyc/elem PER LANE: on [1,512] it
                    # costs 3.3us and head-of-line-blocks the DVE queue.
                    # Reshape both heads' denominators to [128,8] via SBUF
                    # DMAs (row-major pairing, probe-verified) so the recip
                    # uses 128 lanes (~0.1us), then shape back for the
                    # gpsimd partition broadcast.
                    dn8 = rcp.tile([128, 8], F32, tag="dn8")
                    nc.sync.dma_start(out=dn8[0:64, :], in_=avsb0[64:65, :])
                    nc.sync.dma_start(out=dn8[64:128, :], in_=avsb1[64:65, :])
                    rc8 = rcp.tile([128, 8], F32, tag="rc8")
                    with nc.allow_low_precision(reason="softmax normalize bf16"):
                        nc.vector.reciprocal(rc8[:], dn8[:])
                        for h, avt in ((0, avsb0), (1, avsb1)):
                            rc = rcp.tile([1, 512], F32, tag="rcrow")
                            nc.sync.dma_start(out=rc[:],
                                              in_=rc8[64 * h:64 * h + 64, :])
                            bc = bcp.tile([64, 512], F32)
                            nc.gpsimd.partition_broadcast(bc[:], rc[:],
                                                          channels=64)
                            if h == 0:
                                nc.vector.tensor_mul(attnT[0:64, cols],
                                                     avt[0:64, :], bc[:])
                            else:
                                # DVE lanes are partition-locked: odd head's
                                # rows 64-127 via an SBUF bounce + DMA shift
                                tmp = shtmp.tile([64, 512], BF16)
                                nc.vector.tensor_mul(tmp[:], avt[0:64, :], bc[:])
                                nc.sync.dma_start(out=attnT[64:128, cols],
                                                  in_=tmp[:])
                return normalize

            # ================= schedule =================
            # upfront: just enough qkv for attn(0, g0)
            emit_qk_group(3, 0)          # kT pair 0, seq 0-511
            emit_qk_group(0, 0)          # qT pair 0, seq 0-511
            for t in range(4):
                emit_v_tile(t)           # v is all-pairs per tile

            # deadlines: qT(p, g) is read only by group (p, g); kT(p, g') is
            # read by EVERY group (p, g >= g'), so its deadline is the
            # earliest-executing such group - for pair 2 (descending group
            # order) that is the first group of the pair for ALL kT chunks.
            for p in range(N_PAIRS):
                first_step = min(step_base[(p, g)] for g in group_order[p])
                for g in range(NG):
                    if (p, g) == (0, 0):
                        continue
                    kt_dl = min(step_base[(p, gg)] for gg in range(g, NG)) - 1
                    push(kt_dl, lambda m=3 + p, g=g: emit_qk_group(m, g))
                    push(step_base[(p, g)] - 1,
                         lambda m=p, g=g: emit_qk_group(m, g))
            for t in range(4, 16):
                # first pair-0 group reading tile t is g = t//4 (at j = t)
                push(step_base[(0, t // 4)] + t, lambda t=t: emit_v_tile(t))

            # pair-2 groups run DESCENDING so the final (short) g=0 group's
            # proj tiles are the only tail work
            deferred_norm = None
            for pair in range(N_PAIRS):
                for g in group_order[pair]:
                    deferred_norm = emit_attn_group(pair, g,
                                                    pre_unit=deferred_norm)
                    if pair == 2:
                        # proj tiles for this group's columns become valid
                        # once its (deferred) normalize is emitted - which
                        # happens as the NEXT group's pre_unit (step nxt).
                        # deadline nxt+1+LOOKAHEAD => first pullable at
                        # step nxt+1, strictly after that pre_unit.
                        nxt = step_base[(pair, g)] + 4 * g + 4
                        for t in range(4 * g, 4 * g + 4):
                            push(nxt + 1 + LOOKAHEAD,
                                 lambda t=t: emit_proj_tile(t))

            # drain: last group's normalize + anything not pulled + tail proj
            if deferred_norm is not None:
                deferred_norm()
            while work_q:
                work_q.pop(0)[2]()

    nc.compile()
    return nc


def _numpy_fallback(x, mask, W_attn, b_attn, W_proj, b_proj):
    qkv = x @ W_attn + b_attn
    q, k, v = np.split(qkv, 3, axis=-1)

    def heads(t):
        return t.reshape(B, S, N_HEAD, HEAD_DIM).transpose(0, 2, 1, 3)

    q, k, v = heads(q), heads(k), heads(v)
    attn = np.einsum("bhqd,bhkd->bhqk", q, k) / np.sqrt(np.float32(HEAD_DIM))
    attn = attn + mask * (-1e9)
    attn = attn - attn.max(axis=-1, keepdims=True)
    attn = np.exp(attn)
    attn = attn / attn.sum(axis=-1, keepdims=True)
    out = np.einsum("bhqk,bhkd->bhqd", attn, v)
    out = out.transpose(0, 2, 1, 3).reshape(B, S, N_EMBD)
    return (out @ W_proj + b_proj).astype(np.float32)


def make_in_maps(x, W_attn, b_attn, W_proj):
    bf16 = ml_dtypes.bfloat16
    in_maps = []
    for c in range(N_CORES):
        b, hg = divmod(c, 2)
        o = HG_DIM * hg
        in_maps.append({
            "xT": np.ascontiguousarray(x[b].T.astype(bf16)),
            "w_qkv": np.ascontiguousarray(np.concatenate(
                [W_attn[:, o:o + HG_DIM],
                 W_attn[:, 768 + o:768 + o + HG_DIM],
                 W_attn[:, 1536 + o:1536 + o + HG_DIM]], axis=1).astype(bf16)),
            "b_qk": np.ascontiguousarray(np.concatenate(
                [b_attn[o:o + HG_DIM], b_attn[768 + o:768 + o + HG_DIM]])),
            "b_v": np.ascontiguousarray(b_attn[1536 + o:1536 + o + HG_DIM]).astype(bf16),
            "w_proj": np.ascontiguousarray(W_proj[o:o + HG_DIM, :].astype(bf16)),
            "ones": np.ones((1, 128), dtype=bf16),
        })
    return in_maps


def kernel(x, mask, W_attn, b_attn, W_proj, b_proj):
    global LAST_RESULTS, _PROGRAM
    x = np.asarray(x, dtype=np.float32)
    mask = np.asarray(mask, dtype=np.float32)
    W_attn = np.asarray(W_attn, dtype=np.float32)
    b_attn = np.asarray(b_attn, dtype=np.float32)
    W_proj = np.asarray(W_proj, dtype=np.float32)
    b_proj = np.asarray(b_proj, dtype=np.float32)

    # the kernel exploits causal structure; verify the mask actually is causal
    causal = 1.0 - np.tril(np.ones((S, S), dtype=np.float32))
    if mask.shape != (1, 1, S, S) or not np.array_equal(mask[0, 0], causal):
        return _numpy_fallback(x, mask, W_attn, b_attn, W_proj, b_proj)

    from concourse.bass_utils import run_bass_kernel_spmd

    if _PROGRAM is None:
        _PROGRAM = _build_program()

    in_maps = make_in_maps(x, W_attn, b_attn, W_proj)

    trace = bool(int(os.environ.get("ATTN_KERNEL_TRACE", "0")))
    res = run_bass_kernel_spmd(_PROGRAM, in_maps, list(range(N_CORES)), trace=trace)
    LAST_RESULTS = res

    y = np.zeros((B, S, N_EMBD), dtype=np.float32)
    for c in range(N_CORES):
        y[c // 2] += res.results[c]["y"]
    y += b_proj
    return y
